# revision 17
# baseline (speedup 1.0000x reference)
"""Causal self-attention (B=4, T=2048, C=1024, H=16, D=64) on 8 TRN2 cores.

Sharding: core i = (batch b=i//2, head-group g=i%2 of 8 heads).
Each core runs the full pipeline for its (b, g) shard with zero
cross-core communication; the row-parallel out_proj partial sums of the
two head-groups of a batch are added on the host during unsharding
(along with b_out and the v-bias contribution bv @ w_out, which is
linear and precomputed on host).

Compute dtype: bfloat16 operands, fp32 PSUM accumulation.

Key engine-level structure (per core):
  - scores matmuls have K=D=64: the two heads of a feature tile are
    emitted as PE row-tiles (rows 0-63 / 64-127, auto-derived from
    base_partition) so they execute CONCURRENTLY in the PE array.
  - per (t1 slice, t2-block-pair) batch, scores land in two 2-bank
    PSUM tiles sqA/sqB [128, 2(head), 512]; exp runs as one ACT per
    half so the next batch's j0 scores only wait on exp_a (J-stagger,
    keeps the scalar engine dense).
  - the causal mask is applied POST-exp as a multiplicative 0/1 bf16
    mask on E (cheap 2x-rate DVE op, off the ACT critical path). The
    PV lhsT=[v_h | ones] (M=65) accumulates the softmax denominator
    from the masked E, so the denominator stays exact.
  - normalize = copy denominator row, gpsimd partition broadcast,
    single DVE divide into the resident aP tiles.
  - qkv projection / v projection / out_proj matmul chains are
    injected as PE filler paced by a PE-vs-ACT cycle account, so the
    tensor engine stays dense (HAM clock-gate stays warm) while the
    scalar engine is never starved.
"""

import os
import sys

for _p in (
    "/root/.axon_site",
    "/root/.axon_site/_ro/trn_rl_repo",
    "/root/.axon_site/_ro/pypackages",
    "/opt/trn_rl_repo",
):
    if os.path.isdir(_p) and _p not in sys.path:
        sys.path.append(_p)

import numpy as np
import ml_dtypes

import concourse.bass as bass
import concourse.bacc as bacc
import concourse.mybir as mybir
from concourse import tile
from concourse.bass_utils import run_bass_kernel_spmd

BF16NP = ml_dtypes.bfloat16

B, T, C, H, D = 4, 2048, 1024, 16, 64
HPC = 8            # heads per core
GF = HPC * D       # 512: feature width of one head-group
NCORES = 8
KC = C // 128      # 8 contraction tiles over C
NT = T // 128      # 16 time tiles of 128
TS = 512           # t1 slice width
NTS = T // TS      # 4 t1 slices

F32 = mybir.dt.float32
BF16 = mybir.dt.bfloat16
AF = mybir.ActivationFunctionType
ALU = mybir.AluOpType


def build_nc() -> bass.Bass:
    nc = bacc.Bacc()

    # host-preswizzled layouts for large contiguous DMA lines
    xT = nc.declare_dram_parameter("xT", [128, NTS, KC, TS], BF16, isOutput=False)
    wq = nc.declare_dram_parameter("wq", [128, 4, KC, 128], BF16, isOutput=False)
    wk = nc.declare_dram_parameter("wk", [128, 4, KC, 128], BF16, isOutput=False)
    wv = nc.declare_dram_parameter("wv", [128, KC, GF], BF16, isOutput=False)
    wo = nc.declare_dram_parameter("wo", [128, 4, C], BF16, isOutput=False)
    bqc = nc.declare_dram_parameter("bqc", [128, 4], F32, isOutput=False)
    bkc = nc.declare_dram_parameter("bkc", [128, 4], F32, isOutput=False)
    tril01 = nc.declare_dram_parameter("tril01", [128, 128], BF16, isOutput=False)
    out = nc.declare_dram_parameter("out", [T, C], F32, isOutput=True)

    with tile.TileContext(nc) as tc:
        with (
            tc.tile_pool(name="consts", bufs=1) as cpool,
            tc.tile_pool(name="apool", bufs=1) as apool,
        ):
            # v resident in SBUF: [t2-part, t2-tile, head, 64 v + 1 ones]
            v_sb = apool.tile([128, NT, HPC, 65], BF16, tag="vsb")
            nc.vector.memset(v_sb[:, :, :, 64:65], 1.0)
            # attention outputs stay resident in SBUF, feature-major
            aP = [apool.tile([128, T], BF16, tag=f"aP{f}", name=f"aP{f}") for f in range(4)]

            tn01_sb = cpool.tile([128, 2, 2, 128], BF16, tag="tn01")
            bqc_sb = cpool.tile([128, 4], F32, tag="bqc")
            bkc_sb = cpool.tile([128, 4], F32, tag="bkc")

            with (
                tc.tile_pool(name="p1", bufs=1) as p1,
                tc.tile_pool(name="p2", bufs=4) as p2,
                tc.tile_pool(name="pE", bufs=6) as pE,
                tc.tile_pool(name="pN", bufs=2) as pN,
                tc.tile_pool(name="p3s", bufs=6) as p3s,
                tc.tile_pool(name="pp1", bufs=2, space="PSUM") as pp1,
                tc.tile_pool(name="ps2", bufs=1, space="PSUM") as ps2,
                tc.tile_pool(name="pso", bufs=1, space="PSUM") as pso,
            ):
                xT_sb = p1.tile([128, NTS, KC, TS], BF16, tag="xT")
                wq_sb = p1.tile([128, 4, KC, 128], BF16, tag="wq")
                wk_sb = p1.tile([128, 4, KC, 128], BF16, tag="wk")
                wv_sb = p1.tile([128, KC, GF], BF16, tag="wv")
                wo_sb = p1.tile([128, 4, C], BF16, tag="wo")

                # ---- input DMA: few large contiguous transfers,
                # critical-path ones first ----
                nc.sync.dma_start(xT_sb[:, 0, 0:4, :], xT[:, 0, 0:4, :])
                nc.scalar.dma_start(xT_sb[:, 0, 4:8, :], xT[:, 0, 4:8, :])
                nc.scalar.dma_start(wq_sb[:, 0, :, :], wq[:, 0, :, :])
                nc.scalar.dma_start(wk_sb[:, 0, :, :], wk[:, 0, :, :])
                nc.scalar.dma_start(bqc_sb[:, :], bqc[:, :])
                nc.scalar.dma_start(bkc_sb[:, :], bkc[:, :])
                nc.sync.dma_start(wv_sb[:, :, :], wv[:, :, :])
                for a in range(2):
                    for bb in range(2):
                        nc.sync.dma_start(tn01_sb[:, a, bb, :], tril01[:, :])
                for ts in range(1, NTS):
                    nc.sync.dma_start(xT_sb[:, ts, :, :], xT[:, ts, :, :])
                for f in range(1, 4):
                    nc.sync.dma_start(wq_sb[:, f, :, :], wq[:, f, :, :])
                    nc.sync.dma_start(wk_sb[:, f, :, :], wk[:, f, :, :])
                nc.sync.dma_start(wo_sb[:, :, :], wo[:, :, :])

                # all q/k tiles resident (bufs=4)
                qp = {}
                kp = {}
                for f in range(4):
                    qp[f] = p2.tile([128, T], BF16, tag="qp", name="qp")
                    kp[f] = p2.tile([128, T], BF16, tag="kp", name="kp")

                # ---------------- PE-vs-ACT cycle account ----------------
                est = {"pe": 0.0, "act": 0.0}

                def mm_ns(n):
                    return n / 2.4 + 25.0

                # ---------------- PE filler chains ----------------
                def emit_qk_chain(f, ts, which):
                    w_sb, b_sb, dst = (
                        (wq_sb, bqc_sb, qp[f]) if which == "q" else (wk_sb, bkc_sb, kp[f])
                    )
                    acc = pp1.tile([128, TS], F32, tag="acc", name="acc")
                    for k in range(KC):
                        nc.tensor.matmul(
                            acc[:, :],
                            w_sb[:, f, k, :],
                            xT_sb[:, ts, k, :],
                            start=(k == 0),
                            stop=(k == KC - 1),
                        )
                    nc.vector.tensor_scalar_add(
                        dst[:, ts * TS : (ts + 1) * TS], acc[:, :], b_sb[:, f : f + 1]
                    )
                    est["pe"] += KC * mm_ns(TS)

                def emit_v_chain(t):
                    acc = pp1.tile([128, GF], F32, tag="acc", name="acc")
                    for k in range(KC):
                        nc.tensor.matmul(
                            acc[:, :],
                            xT_sb[:, t // 4, k, (t % 4) * 128 : (t % 4 + 1) * 128],
                            wv_sb[:, k, :],
                            start=(k == 0),
                            stop=(k == KC - 1),
                        )
                    nc.vector.tensor_copy(
                        v_sb[:, t, :, 0:64],
                        acc[:, :].rearrange("p (h d) -> p h d", h=HPC),
                    )
                    est["pe"] += KC * mm_ns(GF)

                _stg_tog = [0]

                def emit_outproj_chain(t, n, tail=False):
                    acc3 = pp1.tile([128, TS], F32, tag="acc", name="acc")
                    for ff in range(4):
                        nc.tensor.matmul(
                            acc3[:, :],
                            aP[ff][:, t * 128 : (t + 1) * 128],
                            wo_sb[:, ff, n * TS : (n + 1) * TS],
                            start=(ff == 0),
                            stop=(ff == 3),
                        )
                    stg = p3s.tile([128, TS], F32, tag="stg3", name="stg3")
                    if _stg_tog[0] % 2 == 0:
                        nc.vector.tensor_copy(stg[:, :], acc3[:, :])
                    else:
                        nc.scalar.copy(stg[:, :], acc3[:, :])
                    _stg_tog[0] += 1
                    if tail:
                        dq = (nc.gpsimd, nc.sync, nc.scalar)[_stg_tog[0] % 3]
                    else:
                        dq = nc.gpsimd if _stg_tog[0] % 2 == 0 else nc.sync
                    dq.dma_start(
                        out[t * 128 : (t + 1) * 128, n * TS : (n + 1) * TS], stg[:, :]
                    )
                    est["pe"] += 4 * mm_ns(TS)

                # global filler queue: (marker, closure). markers are
                # (f, t1i, stage): stage 0 = must run before attention body
                # (f, t1i); stage 1 = before that t1i's PV drain.
                filler = []

                def flush_until(marker):
                    while filler and filler[0][0] <= marker:
                        filler.pop(0)[1]()

                def pop_filler(margin_ns=2500.0, max_pops=3):
                    npop = 0
                    while (
                        filler
                        and est["pe"] < est["act"] + margin_ns
                        and npop < max_pops
                    ):
                        filler.pop(0)[1]()
                        npop += 1

                # v chains: tile t needed by attention (0, t//4); tiles 0-3
                # deferred past the first prime (stage 1) so ACT starts early
                for t in range(4):
                    filler.append(((0, 0, 1), lambda t=t: emit_v_chain(t)))
                for t1i in range(1, 4):
                    filler.append(((0, t1i, 0), lambda ts=t1i: emit_qk_chain(0, ts, "q")))
                    filler.append(((0, t1i, 0), lambda ts=t1i: emit_qk_chain(0, ts, "k")))
                    for t in range(4 * t1i, 4 * t1i + 4):
                        filler.append(((0, t1i, 0), lambda t=t: emit_v_chain(t)))
                for f in range(1, 4):
                    for t1i in range(4):
                        filler.append(
                            ((f, t1i, 0), lambda f=f, ts=t1i: emit_qk_chain(f, ts, "q"))
                        )
                        filler.append(
                            ((f, t1i, 0), lambda f=f, ts=t1i: emit_qk_chain(f, ts, "k"))
                        )

                # ---------------- attention ----------------
                for f in range(4):
                    if f == 0:
                        emit_qk_chain(0, 0, "q")
                        emit_qk_chain(0, 0, "k")

                    for t1i in range(NTS):
                        flush_until((f, t1i, 0))
                        t1s = t1i * TS
                        nfull = t1s // 128
                        o_ps = pso.tile([65, 2, TS], F32, tag="outps", name="ops")
                        nmm = [0, 0]
                        batches = [("full", p) for p in range(nfull // 2)]
                        batches += [("diag", 0), ("diag", 1)]

                        def emit_scores_j(b, j, sq):
                            """row-tiled head-pair scores for the j-half."""
                            kind, idx = b
                            if kind == "full":
                                segs = [(2 * idx + j, 0, TS, 0)]
                            elif idx == 0:
                                segs = [(nfull + j, 0, TS - 128 * j, 128 * j)]
                            else:
                                if j == 1:
                                    return
                                segs = [(nfull + 2, 0, 256, 256), (nfull + 3, 256, 128, 384)]
                            for t2, off, nd, qoff in segs:
                                for r in range(2):
                                    nc.tensor.matmul(
                                        sq[:, r, off : off + nd],
                                        kp[f][r * 64 : (r + 1) * 64, t2 * 128 : (t2 + 1) * 128],
                                        qp[f][r * 64 : (r + 1) * 64, t1s + qoff : t1s + TS],
                                        start=True,
                                        stop=True,
                                    )
                                est["pe"] += mm_ns(nd)

                        def exp_widths(b):
                            kind, idx = b
                            if kind == "full":
                                return TS, TS
                            return (TS, 384) if idx == 0 else (384, 0)

                        def emit_exp(b, half, sq, E):
                            w = exp_widths(b)[half]
                            if w > 0:
                                nc.scalar.activation(
                                    E[:, :, 0:w], sq[:, :, 0:w], AF.Exp
                                )
                                est["act"] += (2 * w + 352) / 1.2

                        def emit_emask(b, EA, EB):
                            kind, idx = b
                            if kind != "diag":
                                return
                            if idx == 0:
                                # diagonal 128-blocks at local col 0 of both halves
                                for E in (EA, EB):
                                    ap = E[:, :, 0:128]
                                    nc.vector.tensor_mul(ap, ap, tn01_sb[:, :, 0, :])
                            else:
                                ap = EA[:, :, 0:512].rearrange(
                                    "p r (d x) -> p r d x", d=2
                                )[:, :, :, 0:128]
                                nc.vector.tensor_mul(ap, ap, tn01_sb[:, :, :, :])

                        def emit_pv(b, EA, EB):
                            kind, idx = b
                            if kind == "full":
                                segs = [(2 * idx, EA, 0, TS, 0), (2 * idx + 1, EB, 0, TS, 0)]
                            elif idx == 0:
                                segs = [(nfull, EA, 0, TS, 0), (nfull + 1, EB, 0, 384, 128)]
                            else:
                                segs = [
                                    (nfull + 2, EA, 0, 256, 256),
                                    (nfull + 3, EA, 256, 128, 384),
                                ]
                            last = kind == "diag" and idx == 1
                            for r in range(2):
                                for si, (t2, E, eoff, nd, ocol) in enumerate(segs):
                                    nc.tensor.matmul(
                                        o_ps[:, r, ocol : ocol + nd],
                                        v_sb[:, t2, 2 * f + r, :],
                                        E[:, r, eoff : eoff + nd],
                                        start=(nmm[r] == 0),
                                        stop=(last and si == len(segs) - 1),
                                    )
                                    nmm[r] += 1
                                    est["pe"] += mm_ns(nd)

                        # software pipeline over batches
                        pvq = []
                        sqA = ps2.tile([128, 2, TS], F32, tag="sqA", name="sqA")
                        sqB = ps2.tile([128, 2, TS], F32, tag="sqB", name="sqB")
                        emit_scores_j(batches[0], 0, sqA)
                        emit_scores_j(batches[0], 1, sqB)
                        cur = (sqA, sqB)
                        for i, b in enumerate(batches):
                            EA = pE.tile([128, 2, TS], BF16, tag="EA", name="EA")
                            EB = pE.tile([128, 2, TS], BF16, tag="EB", name="EB")
                            emit_exp(b, 0, cur[0], EA)
                            emit_exp(b, 1, cur[1], EB)
                            emit_emask(b, EA, EB)
                            if len(pvq) >= 3:
                                emit_pv(*pvq.pop(0))
                            pop_filler()
                            if i + 1 < len(batches):
                                nA = ps2.tile([128, 2, TS], F32, tag="sqA", name="sqA")
                                nB = ps2.tile([128, 2, TS], F32, tag="sqB", name="sqB")
                                emit_scores_j(batches[i + 1], 0, nA)
                                emit_scores_j(batches[i + 1], 1, nB)
                                cur = (nA, nB)
                            pvq.append((b, EA, EB))
                        flush_until((f, t1i, 1))
                        for b, EA, EB in pvq:
                            emit_pv(b, EA, EB)
                        pop_filler()

                        # normalize rows 0-63 by row 64 (softmax denominator)
                        # NB: DVE ops with partition-shifted PSUM sources must
                        # go through a copy first (HW AP lowering bug).
                        dn = pN.tile([1, 2, TS], F32, tag="dn", name="dn")
                        nc.vector.tensor_copy(dn[:, :, :], o_ps[64:65, :, :])
                        bc_sb = pN.tile([64, 2, TS], F32, tag="bc", name="bc")
                        nc.gpsimd.partition_broadcast(bc_sb[:, :, :], dn[:, :, :])
                        rc_sb = pN.tile([64, 2, TS], F32, tag="rc", name="rc")
                        nc.vector.reciprocal_approx_fast(rc_sb[:, :, :], bc_sb[:, :, :])
                        for r in range(2):
                            nc.vector.tensor_mul(
                                aP[f][r * 64 : (r + 1) * 64, t1s : t1s + TS],
                                o_ps[0:64, r, :],
                                rc_sb[:, r, :],
                            )
                        if f == 3:
                            for t in range(t1s // 128, t1s // 128 + 4):
                                for n in range(2):
                                    filler.append(
                                        (
                                            (9, 9, 9),
                                            lambda t=t, n=n: emit_outproj_chain(t, n),
                                        )
                                    )
                            pop_filler()
                # drain all remaining filler (tail out_proj chains)
                flush_until((9, 9, 9))

    nc.finalize()
    return nc


def make_in_maps(x, w_qkv, b_qkv, w_out, b_out):
    x = np.asarray(x, dtype=np.float32)
    w_qkv = np.asarray(w_qkv, dtype=np.float32)
    b_qkv = np.asarray(b_qkv, dtype=np.float32)
    w_out = np.asarray(w_out, dtype=np.float32)

    def bf(a):
        return np.ascontiguousarray(a).astype(BF16NP)

    scale = 1.0 / np.sqrt(D)
    # multiplicative causal mask for diagonal blocks (applied post-exp)
    tril01 = np.tril(np.ones((128, 128), dtype=np.float32)).T

    # pre-swizzles for large contiguous DMA lines
    def sw_x(xb):  # [T, C] -> [128, NTS, KC, TS]
        return xb.T.reshape(KC, 128, NTS, TS).transpose(1, 2, 0, 3)

    def sw_w(w):  # [C, GF] -> [128, 4, KC, 128]
        return w.reshape(KC, 128, 4, 128).transpose(1, 2, 0, 3)

    def sw_wv(w):  # [C, GF] -> [128, KC, GF]
        return w.reshape(KC, 128, GF).transpose(1, 0, 2)

    def sw_wo(w):  # [GF, C] -> [128, 4, C]
        return w.reshape(4, 128, C).transpose(1, 0, 2)

    in_maps = []
    for core in range(NCORES):
        b, g = core // 2, core % 2
        sl = slice(g * GF, (g + 1) * GF)
        bq = (b_qkv[sl] * scale).reshape(4, 128).T  # [128, 4] per-feat col
        bk = b_qkv[C + g * GF : C + (g + 1) * GF].reshape(4, 128).T
        in_maps.append(
            {
                "xT": bf(sw_x(x[b])),
                "wq": bf(sw_w(w_qkv[:, sl] * scale)),
                "wk": bf(sw_w(w_qkv[:, C + g * GF : C + (g + 1) * GF])),
                "wv": bf(sw_wv(w_qkv[:, 2 * C + g * GF : 2 * C + (g + 1) * GF])),
                "wo": bf(sw_wo(w_out[sl, :])),
                "bqc": np.ascontiguousarray(bq, dtype=np.float32),
                "bkc": np.ascontiguousarray(bk, dtype=np.float32),
                "tril01": bf(tril01),
            }
        )
    return in_maps


_NC_CACHE = {}


def run(inputs: dict, trace: bool = False):
    """Compile (cached) + run on 8 cores. Returns (full_output, BassKernelResults)."""
    if "nc" not in _NC_CACHE:
        _NC_CACHE["nc"] = build_nc()
    nc = _NC_CACHE["nc"]
    in_maps = make_in_maps(**inputs)
    res = run_bass_kernel_spmd(
        nc, in_maps, core_ids=list(range(NCORES)), trace=trace
    )
    outs = [np.asarray(m["out"], dtype=np.float32) for m in res.results]
    full = np.stack([outs[2 * b] + outs[2 * b + 1] for b in range(B)], axis=0)
    # host bias: b_out plus the (linear) v-bias contribution bv @ w_out
    b_qkv = np.asarray(inputs["b_qkv"], dtype=np.float32)
    w_out = np.asarray(inputs["w_out"], dtype=np.float32)
    bv = b_qkv[2 * C :]
    full += np.asarray(inputs["b_out"], dtype=np.float32) + bv @ w_out
    return full, res


def kernel(**inputs) -> np.ndarray:
    full, _ = run(inputs, trace=False)
    return full


# revision 20
# speedup vs baseline: 1.0107x; 1.0107x over previous
"""Causal self-attention (B=4, T=2048, C=1024, H=16, D=64) on 8 TRN2 cores.

Sharding: core i = (batch b=i//2, head-group g=i%2 of 8 heads).
Each core runs the full pipeline for its (b, g) shard with zero
cross-core communication; the row-parallel out_proj partial sums of the
two head-groups of a batch are added on the host during unsharding
(along with b_out and the v-bias contribution bv @ w_out, which is
linear and precomputed on host).

Compute dtype: bfloat16 operands, fp32 PSUM accumulation.

Key engine-level structure (per core):
  - scores matmuls have K=D=64: the two heads of a feature tile are
    emitted as PE row-tiles (rows 0-63 / 64-127, auto-derived from
    base_partition) so they execute CONCURRENTLY in the PE array.
  - per (t1 slice, t2-block-pair) batch, scores land in two 2-bank
    PSUM tiles sqA/sqB [128, 2(head), 512]; exp runs as one ACT per
    half so the next batch's j0 scores only wait on exp_a (J-stagger,
    keeps the scalar engine dense).
  - the causal mask is applied POST-exp as a multiplicative 0/1 bf16
    mask on E (cheap 2x-rate DVE op, off the ACT critical path). The
    PV lhsT=[v_h | ones] (M=65) accumulates the softmax denominator
    from the masked E, so the denominator stays exact.
  - normalize = copy denominator row, gpsimd partition broadcast,
    single DVE divide into the resident aP tiles.
  - qkv projection / v projection / out_proj matmul chains are
    injected as PE filler paced by a PE-vs-ACT cycle account, so the
    tensor engine stays dense (HAM clock-gate stays warm) while the
    scalar engine is never starved.
"""

import os
import sys

for _p in (
    "/root/.axon_site",
    "/root/.axon_site/_ro/trn_rl_repo",
    "/root/.axon_site/_ro/pypackages",
    "/opt/trn_rl_repo",
):
    if os.path.isdir(_p) and _p not in sys.path:
        sys.path.append(_p)

import numpy as np
import ml_dtypes

import concourse.bass as bass
import concourse.bacc as bacc
import concourse.mybir as mybir
from concourse import tile
from concourse.bass_utils import run_bass_kernel_spmd

BF16NP = ml_dtypes.bfloat16

B, T, C, H, D = 4, 2048, 1024, 16, 64
HPC = 8            # heads per core
GF = HPC * D       # 512: feature width of one head-group
NCORES = 8
KC = C // 128      # 8 contraction tiles over C
NT = T // 128      # 16 time tiles of 128
TS = 512           # t1 slice width
NTS = T // TS      # 4 t1 slices

F32 = mybir.dt.float32
BF16 = mybir.dt.bfloat16
AF = mybir.ActivationFunctionType
ALU = mybir.AluOpType


def build_nc() -> bass.Bass:
    nc = bacc.Bacc()

    # host-preswizzled layouts for large contiguous DMA lines
    xT = nc.declare_dram_parameter("xT", [128, NTS, KC, TS], BF16, isOutput=False)
    wq = nc.declare_dram_parameter("wq", [128, 4, KC, 128], BF16, isOutput=False)
    wk = nc.declare_dram_parameter("wk", [128, 4, KC, 128], BF16, isOutput=False)
    wv = nc.declare_dram_parameter("wv", [128, KC, GF], BF16, isOutput=False)
    wo = nc.declare_dram_parameter("wo", [128, 4, C], BF16, isOutput=False)
    bqc = nc.declare_dram_parameter("bqc", [128, 4], F32, isOutput=False)
    bkc = nc.declare_dram_parameter("bkc", [128, 4], F32, isOutput=False)
    tril01 = nc.declare_dram_parameter("tril01", [128, 128], BF16, isOutput=False)
    out = nc.declare_dram_parameter("out", [T, C], F32, isOutput=True)

    with tile.TileContext(nc) as tc:
        with (
            tc.tile_pool(name="consts", bufs=1) as cpool,
            tc.tile_pool(name="apool", bufs=1) as apool,
        ):
            # v resident in SBUF: [t2-part, t2-tile, head, 64 v + 1 ones]
            v_sb = apool.tile([128, NT, HPC, 65], BF16, tag="vsb")
            nc.vector.memset(v_sb[:, :, :, 64:65], 1.0)
            # attention outputs stay resident in SBUF, feature-major
            aP = [apool.tile([128, T], BF16, tag=f"aP{f}", name=f"aP{f}") for f in range(4)]

            tn01_sb = cpool.tile([128, 2, 2, 128], BF16, tag="tn01")
            bqc_sb = cpool.tile([128, 4], F32, tag="bqc")
            bkc_sb = cpool.tile([128, 4], F32, tag="bkc")

            with (
                tc.tile_pool(name="p1", bufs=1) as p1,
                tc.tile_pool(name="p2", bufs=4) as p2,
                tc.tile_pool(name="pE", bufs=6) as pE,
                tc.tile_pool(name="pN", bufs=3) as pN,
                tc.tile_pool(name="p3s", bufs=8) as p3s,
                tc.tile_pool(name="pp1", bufs=2, space="PSUM") as pp1,
                tc.tile_pool(name="ps2", bufs=1, space="PSUM") as ps2,
                tc.tile_pool(name="pso", bufs=1, space="PSUM") as pso,
            ):
                xT_sb = p1.tile([128, NTS, KC, TS], BF16, tag="xT")
                wq_sb = p1.tile([128, 4, KC, 128], BF16, tag="wq")
                wk_sb = p1.tile([128, 4, KC, 128], BF16, tag="wk")
                wv_sb = p1.tile([128, KC, GF], BF16, tag="wv")
                wo_sb = p1.tile([128, 4, C], BF16, tag="wo")

                # ---- input DMA: few large contiguous transfers,
                # critical-path ones first ----
                nc.sync.dma_start(xT_sb[:, 0, :, :], xT[:, 0, :, :])
                nc.sync.dma_start(wq_sb[:, 0, :, :], wq[:, 0, :, :])
                nc.sync.dma_start(wk_sb[:, 0, :, :], wk[:, 0, :, :])
                nc.sync.dma_start(bqc_sb[:, :], bqc[:, :])
                nc.sync.dma_start(bkc_sb[:, :], bkc[:, :])
                nc.sync.dma_start(wv_sb[:, :, :], wv[:, :, :])
                for a in range(2):
                    for bb in range(2):
                        nc.sync.dma_start(tn01_sb[:, a, bb, :], tril01[:, :])
                for ts in range(1, NTS):
                    nc.sync.dma_start(xT_sb[:, ts, :, :], xT[:, ts, :, :])
                for f in range(1, 4):
                    nc.sync.dma_start(wq_sb[:, f, :, :], wq[:, f, :, :])
                    nc.sync.dma_start(wk_sb[:, f, :, :], wk[:, f, :, :])
                nc.sync.dma_start(wo_sb[:, :, :], wo[:, :, :])

                # all q/k tiles resident (bufs=4)
                qp = {}
                kp = {}
                for f in range(4):
                    qp[f] = p2.tile([128, T], BF16, tag="qp", name="qp")
                    kp[f] = p2.tile([128, T], BF16, tag="kp", name="kp")

                # ---------------- PE-vs-ACT cycle account ----------------
                est = {"pe": 0.0, "act": 0.0}

                def mm_ns(n):
                    return n / 2.4 + 25.0

                # ---------------- PE filler chains ----------------
                def emit_qk_chain(f, ts, which):
                    w_sb, b_sb, dst = (
                        (wq_sb, bqc_sb, qp[f]) if which == "q" else (wk_sb, bkc_sb, kp[f])
                    )
                    acc = pp1.tile([128, TS], F32, tag="acc", name="acc")
                    for k in range(KC):
                        nc.tensor.matmul(
                            acc[:, :],
                            w_sb[:, f, k, :],
                            xT_sb[:, ts, k, :],
                            start=(k == 0),
                            stop=(k == KC - 1),
                        )
                    nc.vector.tensor_scalar_add(
                        dst[:, ts * TS : (ts + 1) * TS], acc[:, :], b_sb[:, f : f + 1]
                    )
                    est["pe"] += KC * mm_ns(TS)

                def emit_v_chain(t):
                    acc = pp1.tile([128, GF], F32, tag="acc", name="acc")
                    for k in range(KC):
                        nc.tensor.matmul(
                            acc[:, :],
                            xT_sb[:, t // 4, k, (t % 4) * 128 : (t % 4 + 1) * 128],
                            wv_sb[:, k, :],
                            start=(k == 0),
                            stop=(k == KC - 1),
                        )
                    nc.vector.tensor_copy(
                        v_sb[:, t, :, 0:64],
                        acc[:, :].rearrange("p (h d) -> p h d", h=HPC),
                    )
                    est["pe"] += KC * mm_ns(GF)

                _stg_tog = [0]

                def emit_outproj_chain(t, n, tail=False):
                    acc3 = pp1.tile([128, TS], F32, tag="acc", name="acc")
                    for ff in range(4):
                        nc.tensor.matmul(
                            acc3[:, :],
                            aP[ff][:, t * 128 : (t + 1) * 128],
                            wo_sb[:, ff, n * TS : (n + 1) * TS],
                            start=(ff == 0),
                            stop=(ff == 3),
                        )
                    stg = p3s.tile([128, TS], F32, tag="stg3", name="stg3")
                    if _stg_tog[0] % 2 == 0:
                        nc.vector.tensor_copy(stg[:, :], acc3[:, :])
                    else:
                        nc.scalar.copy(stg[:, :], acc3[:, :])
                    _stg_tog[0] += 1
                    dq = nc.gpsimd if _stg_tog[0] % 2 == 0 else nc.sync
                    dq.dma_start(
                        out[t * 128 : (t + 1) * 128, n * TS : (n + 1) * TS], stg[:, :]
                    )
                    est["pe"] += 4 * mm_ns(TS)

                # global filler queue: (marker, closure). markers are
                # (f, t1i, stage): stage 0 = must run before attention body
                # (f, t1i); stage 1 = before that t1i's PV drain.
                filler = []

                def flush_until(marker):
                    while filler and filler[0][0] <= marker:
                        filler.pop(0)[1]()

                def pop_filler(margin_ns=2500.0, max_pops=3):
                    npop = 0
                    while (
                        filler
                        and est["pe"] < est["act"] + margin_ns
                        and npop < max_pops
                    ):
                        filler.pop(0)[1]()
                        npop += 1

                # v chains: tile t needed by attention (0, t//4); tiles 0-3
                # deferred past the first prime (stage 1) so ACT starts early
                for t in range(4):
                    filler.append(((0, 0, 1), lambda t=t: emit_v_chain(t)))
                for t1i in range(1, 4):
                    filler.append(((0, t1i, 0), lambda ts=t1i: emit_qk_chain(0, ts, "q")))
                    filler.append(((0, t1i, 0), lambda ts=t1i: emit_qk_chain(0, ts, "k")))
                    for t in range(4 * t1i, 4 * t1i + 4):
                        filler.append(((0, t1i, 0), lambda t=t: emit_v_chain(t)))
                for f in range(1, 4):
                    for t1i in range(4):
                        filler.append(
                            ((f, t1i, 0), lambda f=f, ts=t1i: emit_qk_chain(f, ts, "q"))
                        )
                        filler.append(
                            ((f, t1i, 0), lambda f=f, ts=t1i: emit_qk_chain(f, ts, "k"))
                        )

                # ---------------- attention ----------------
                for f in range(4):
                    if f == 0:
                        emit_qk_chain(0, 0, "q")
                        emit_qk_chain(0, 0, "k")

                    for t1i in range(NTS):
                        flush_until((f, t1i, 0))
                        t1s = t1i * TS
                        nfull = t1s // 128
                        o_ps = pso.tile([65, 2, TS], F32, tag="outps", name="ops")
                        nmm = [0, 0]
                        batches = [("full", p) for p in range(nfull // 2)]
                        batches += [("diag", 0), ("diag", 1)]

                        def emit_scores_j(b, j, sq):
                            """row-tiled head-pair scores for the j-half."""
                            kind, idx = b
                            if kind == "full":
                                segs = [(2 * idx + j, 0, TS, 0)]
                            elif idx == 0:
                                segs = [(nfull + j, 0, TS - 128 * j, 128 * j)]
                            else:
                                if j == 1:
                                    return
                                segs = [(nfull + 2, 0, 256, 256), (nfull + 3, 256, 128, 384)]
                            for t2, off, nd, qoff in segs:
                                for r in range(2):
                                    nc.tensor.matmul(
                                        sq[:, r, off : off + nd],
                                        kp[f][r * 64 : (r + 1) * 64, t2 * 128 : (t2 + 1) * 128],
                                        qp[f][r * 64 : (r + 1) * 64, t1s + qoff : t1s + TS],
                                        start=True,
                                        stop=True,
                                    )
                                est["pe"] += mm_ns(nd)

                        def exp_widths(b):
                            kind, idx = b
                            if kind == "full":
                                return TS, TS
                            return (TS, 384) if idx == 0 else (384, 0)

                        def emit_exp(b, half, sq, E):
                            w = exp_widths(b)[half]
                            if w > 0:
                                nc.scalar.activation(
                                    E[:, :, 0:w], sq[:, :, 0:w], AF.Exp
                                )
                                est["act"] += (2 * w + 352) / 1.2

                        def emit_emask(b, EA, EB):
                            kind, idx = b
                            if kind != "diag":
                                return
                            if idx == 0:
                                # diagonal 128-blocks at local col 0 of both halves
                                for E in (EA, EB):
                                    ap = E[:, :, 0:128]
                                    nc.vector.tensor_mul(ap, ap, tn01_sb[:, :, 0, :])
                            else:
                                ap = EA[:, :, 0:512].rearrange(
                                    "p r (d x) -> p r d x", d=2
                                )[:, :, :, 0:128]
                                nc.vector.tensor_mul(ap, ap, tn01_sb[:, :, :, :])

                        def emit_pv(b, EA, EB):
                            kind, idx = b
                            if kind == "full":
                                segs = [(2 * idx, EA, 0, TS, 0), (2 * idx + 1, EB, 0, TS, 0)]
                            elif idx == 0:
                                segs = [(nfull, EA, 0, TS, 0), (nfull + 1, EB, 0, 384, 128)]
                            else:
                                segs = [
                                    (nfull + 2, EA, 0, 256, 256),
                                    (nfull + 3, EA, 256, 128, 384),
                                ]
                            last = kind == "diag" and idx == 1
                            for r in range(2):
                                for si, (t2, E, eoff, nd, ocol) in enumerate(segs):
                                    nc.tensor.matmul(
                                        o_ps[:, r, ocol : ocol + nd],
                                        v_sb[:, t2, 2 * f + r, :],
                                        E[:, r, eoff : eoff + nd],
                                        start=(nmm[r] == 0),
                                        stop=(last and si == len(segs) - 1),
                                    )
                                    nmm[r] += 1
                                    est["pe"] += mm_ns(nd)

                        # software pipeline over batches
                        pvq = []
                        sqA = ps2.tile([128, 2, TS], F32, tag="sqA", name="sqA")
                        sqB = ps2.tile([128, 2, TS], F32, tag="sqB", name="sqB")
                        emit_scores_j(batches[0], 0, sqA)
                        emit_scores_j(batches[0], 1, sqB)
                        cur = (sqA, sqB)
                        for i, b in enumerate(batches):
                            EA = pE.tile([128, 2, TS], BF16, tag="EA", name="EA")
                            EB = pE.tile([128, 2, TS], BF16, tag="EB", name="EB")
                            emit_exp(b, 0, cur[0], EA)
                            emit_exp(b, 1, cur[1], EB)
                            emit_emask(b, EA, EB)
                            if len(pvq) >= 3:
                                emit_pv(*pvq.pop(0))
                            pop_filler()
                            if i + 1 < len(batches):
                                nA = ps2.tile([128, 2, TS], F32, tag="sqA", name="sqA")
                                nB = ps2.tile([128, 2, TS], F32, tag="sqB", name="sqB")
                                emit_scores_j(batches[i + 1], 0, nA)
                                emit_scores_j(batches[i + 1], 1, nB)
                                cur = (nA, nB)
                            pvq.append((b, EA, EB))
                        flush_until((f, t1i, 1))
                        for b, EA, EB in pvq:
                            emit_pv(b, EA, EB)
                        pop_filler()

                        # normalize rows 0-63 by row 64 (softmax denominator)
                        # NB: DVE ops with partition-shifted PSUM sources must
                        # go through a copy first (HW AP lowering bug).
                        dn = pN.tile([1, 2, TS], F32, tag="dn", name="dn")
                        nc.vector.tensor_copy(dn[:, :, :], o_ps[64:65, :, :])
                        bc_sb = pN.tile([64, 2, TS], F32, tag="bc", name="bc")
                        nc.gpsimd.partition_broadcast(bc_sb[:, :, :], dn[:, :, :])
                        rc_sb = pN.tile([64, 2, TS], F32, tag="rc", name="rc")
                        nc.vector.reciprocal_approx_fast(rc_sb[:, :, :], bc_sb[:, :, :])
                        for r in range(2):
                            nc.vector.tensor_mul(
                                aP[f][r * 64 : (r + 1) * 64, t1s : t1s + TS],
                                o_ps[0:64, r, :],
                                rc_sb[:, r, :],
                            )
                        if f == 3:
                            for t in range(t1s // 128, t1s // 128 + 4):
                                for n in range(2):
                                    filler.append(
                                        (
                                            (9, 9, 9),
                                            lambda t=t, n=n: emit_outproj_chain(t, n),
                                        )
                                    )
                            pop_filler()
                # drain all remaining filler (tail out_proj chains)
                flush_until((9, 9, 9))

    nc.finalize()
    return nc


def make_in_maps(x, w_qkv, b_qkv, w_out, b_out):
    x = np.asarray(x, dtype=np.float32)
    w_qkv = np.asarray(w_qkv, dtype=np.float32)
    b_qkv = np.asarray(b_qkv, dtype=np.float32)
    w_out = np.asarray(w_out, dtype=np.float32)

    def bf(a):
        return np.ascontiguousarray(a).astype(BF16NP)

    scale = 1.0 / np.sqrt(D)
    # multiplicative causal mask for diagonal blocks (applied post-exp)
    tril01 = np.tril(np.ones((128, 128), dtype=np.float32)).T

    # pre-swizzles for large contiguous DMA lines
    def sw_x(xb):  # [T, C] -> [128, NTS, KC, TS]
        return xb.T.reshape(KC, 128, NTS, TS).transpose(1, 2, 0, 3)

    def sw_w(w):  # [C, GF] -> [128, 4, KC, 128]
        return w.reshape(KC, 128, 4, 128).transpose(1, 2, 0, 3)

    def sw_wv(w):  # [C, GF] -> [128, KC, GF]
        return w.reshape(KC, 128, GF).transpose(1, 0, 2)

    def sw_wo(w):  # [GF, C] -> [128, 4, C]
        return w.reshape(4, 128, C).transpose(1, 0, 2)

    in_maps = []
    for core in range(NCORES):
        b, g = core // 2, core % 2
        sl = slice(g * GF, (g + 1) * GF)
        bq = (b_qkv[sl] * scale).reshape(4, 128).T  # [128, 4] per-feat col
        bk = b_qkv[C + g * GF : C + (g + 1) * GF].reshape(4, 128).T
        in_maps.append(
            {
                "xT": bf(sw_x(x[b])),
                "wq": bf(sw_w(w_qkv[:, sl] * scale)),
                "wk": bf(sw_w(w_qkv[:, C + g * GF : C + (g + 1) * GF])),
                "wv": bf(sw_wv(w_qkv[:, 2 * C + g * GF : 2 * C + (g + 1) * GF])),
                "wo": bf(sw_wo(w_out[sl, :])),
                "bqc": np.ascontiguousarray(bq, dtype=np.float32),
                "bkc": np.ascontiguousarray(bk, dtype=np.float32),
                "tril01": bf(tril01),
            }
        )
    return in_maps


_NC_CACHE = {}


def run(inputs: dict, trace: bool = False):
    """Compile (cached) + run on 8 cores. Returns (full_output, BassKernelResults)."""
    if "nc" not in _NC_CACHE:
        _NC_CACHE["nc"] = build_nc()
    nc = _NC_CACHE["nc"]
    in_maps = make_in_maps(**inputs)
    res = run_bass_kernel_spmd(
        nc, in_maps, core_ids=list(range(NCORES)), trace=trace
    )
    outs = [np.asarray(m["out"], dtype=np.float32) for m in res.results]
    full = np.stack([outs[2 * b] + outs[2 * b + 1] for b in range(B)], axis=0)
    # host bias: b_out plus the (linear) v-bias contribution bv @ w_out
    b_qkv = np.asarray(inputs["b_qkv"], dtype=np.float32)
    w_out = np.asarray(inputs["w_out"], dtype=np.float32)
    bv = b_qkv[2 * C :]
    full += np.asarray(inputs["b_out"], dtype=np.float32) + bv @ w_out
    return full, res


def kernel(**inputs) -> np.ndarray:
    full, _ = run(inputs, trace=False)
    return full
